# revision 8
# baseline (speedup 1.0000x reference)
"""Trainium2 Bass kernel for the scatter-memory transformer block.

Computation (see harness reference):
    ep_w  = softmax(x @ We.T + be)   over 65536 slots
    episodic = ep_w @ ep_mem
    sem_w = softmax(x @ Ws.T + bs)   over 131072 slots
    semantic = sem_w @ sem_mem
    out = concat([episodic, x]) @ Wc.T + bc
    return (out, semantic)

Strategy: shard the slot axis across 8 NeuronCores (sequence-parallel flash
cross-attention over a fixed KV set).  Each core streams its slot shard once
from HBM (memory-bound: 192 MB/core), computing
    p[e, t]   = exp(W[e] . x[t] + b[e])          (no max subtraction -- logits
                                                  are O(0.2) for this model)
    part[t,h] = sum_e p[e, t] * mem[e, h]        (PSUM accumulation)
    s[t]      = sum_e p[e, t]
The host sums the 8 unnormalized partials, normalizes, and applies the tiny
consolidation linear.
"""

import numpy as np

import concourse.bass as bass
import concourse.mybir as mybir
import concourse.tile as tile
from concourse import bacc
from concourse.bass_utils import run_bass_kernel_spmd

# Problem dims (hardcoded per harness contract).
B, S, H = 2, 128, 1024
T = B * S  # 256 query tokens
EP, SEM = 65536, 131072
NCORES = 8
EP_SH = EP // NCORES  # 8192 episodic slots per core
SEM_SH = SEM // NCORES  # 16384 semantic slots per core
KH = H // 128  # 8 contraction chunks of 128

F32 = mybir.dt.float32

# Precision of the streamed operands (projections, memory banks, x, p):
# "fp32" is exact-ish (~1e-6 rel err); "fp16" halves HBM traffic
# (~2e-4 rel err from input quantization; accumulation stays fp32 in PSUM).
STREAM_DT = "fp32"
_CFG = {
    "fp32": (mybir.dt.float32, np.float32, 512),
    "fp16": (mybir.dt.float16, np.float16, 1024),
}


def _build_bass():
    SDT, _, CHUNK = _CFG[STREAM_DT]
    nc = bacc.Bacc(
        "TRN2",
        target_bir_lowering=False,
        debug=False,
        num_devices=NCORES,
    )

    xT_d = nc.dram_tensor("xT", [H, T], SDT, kind="ExternalInput")
    weT_d = nc.dram_tensor("weT", [H, EP_SH], SDT, kind="ExternalInput")
    be_d = nc.dram_tensor("be", [EP_SH], F32, kind="ExternalInput")
    epm_d = nc.dram_tensor("epm", [EP_SH, H], SDT, kind="ExternalInput")
    wsT_d = nc.dram_tensor("wsT", [H, SEM_SH], SDT, kind="ExternalInput")
    bs_d = nc.dram_tensor("bs", [SEM_SH], F32, kind="ExternalInput")
    smm_d = nc.dram_tensor("smm", [SEM_SH, H], SDT, kind="ExternalInput")

    epo_d = nc.dram_tensor("ep_part", [T, H], F32, kind="ExternalOutput")
    eps_d = nc.dram_tensor("ep_s", [1, T], F32, kind="ExternalOutput")
    smo_d = nc.dram_tensor("sem_part", [T, H], F32, kind="ExternalOutput")
    sms_d = nc.dram_tensor("sem_s", [1, T], F32, kind="ExternalOutput")

    with tile.TileContext(nc) as tc:
        with (
            tc.tile_pool(name="const", bufs=1) as cpool,
            tc.tile_pool(name="wstream", bufs=2) as wpool,
            tc.tile_pool(name="mstream", bufs=2) as mpool,
            tc.tile_pool(name="ptile", bufs=3) as ppool,
            tc.tile_pool(name="outp", bufs=2) as opool,
            tc.tile_pool(name="acc", bufs=1, space="PSUM") as acc_pool,
            tc.tile_pool(name="lg", bufs=2, space="PSUM") as lg_pool,
            tc.tile_pool(name="sm", bufs=1, space="PSUM") as sm_pool,
        ):
            # x.T resident in SBUF, laid out [p, k, t] with h = k*128 + p.
            xT_sb = cpool.tile([128, KH, T], SDT)
            nc.sync.dma_start(out=xT_sb, in_=xT_d[:, :].rearrange("(k p) t -> p k t", p=128))
            ones_sb = cpool.tile([128, 1], SDT)
            nc.vector.memset(ones_sb, 1.0)
            # Per-slot biases, [p, j] with slot = j*128 + p.
            be_sb = cpool.tile([128, EP_SH // 128], F32)
            nc.sync.dma_start(out=be_sb, in_=be_d[:].rearrange("(j p) -> p j", p=128))
            bs_sb = cpool.tile([128, SEM_SH // 128], F32)
            nc.sync.dma_start(out=bs_sb, in_=bs_d[:].rearrange("(j p) -> p j", p=128))

            def phase(n_sh, wT_d, mem_d, b_sb, out_d, s_out_d, pfx):
                n_chunks = n_sh // CHUNK
                jc = CHUNK // 128  # 128-slot subtiles per chunk
                accs = [
                    [
                        acc_pool.tile([128, 512], F32, tag=f"acc{th}{hh}", name=f"{pfx}acc{th}{hh}")
                        for hh in range(2)
                    ]
                    for th in range(2)
                ]
                s_ps = sm_pool.tile([1, T], F32, tag="sums", name=f"{pfx}sums")

                for c in range(n_chunks):
                    e0 = c * CHUNK
                    wt = wpool.tile([128, KH, CHUNK], SDT, tag="wt", name=f"{pfx}wt{c}")
                    nc.sync.dma_start(
                        out=wt, in_=wT_d[:, e0 : e0 + CHUNK].rearrange("(k p) e -> p k e", p=128)
                    )
                    mm = mpool.tile([128, jc, H], SDT, tag="mm", name=f"{pfx}mm{c}")
                    nc.sync.dma_start(
                        out=mm, in_=mem_d[e0 : e0 + CHUNK, :].rearrange("(j p) h -> p j h", p=128)
                    )
                    for j in range(jc):
                        # logits tile [128 slots, 256 tokens]
                        lp = lg_pool.tile([128, T], F32, tag="lg", name=f"{pfx}lg{c}_{j}")
                        for k in range(KH):
                            nc.tensor.matmul(
                                lp,
                                wt[:, k, j * 128 : (j + 1) * 128],
                                xT_sb[:, k, :],
                                start=(k == 0),
                                stop=(k == KH - 1),
                            )
                        p_sb = ppool.tile([128, T], SDT, tag="p", name=f"{pfx}p{c}_{j}")
                        gj = e0 // 128 + j
                        nc.scalar.activation(
                            out=p_sb,
                            in_=lp,
                            func=mybir.ActivationFunctionType.Exp,
                            bias=b_sb[:, gj : gj + 1],
                            scale=1.0,
                        )
                        first = c == 0 and j == 0
                        last = c == n_chunks - 1 and j == jc - 1
                        for th in range(2):
                            for hh in range(2):
                                nc.tensor.matmul(
                                    accs[th][hh],
                                    p_sb[:, th * 128 : (th + 1) * 128],
                                    mm[:, j, hh * 512 : (hh + 1) * 512],
                                    start=first,
                                    stop=last,
                                )
                        nc.tensor.matmul(s_ps, ones_sb, p_sb, start=first, stop=last)

                for th in range(2):
                    o_sb = opool.tile([128, H], F32, tag=f"o{th}", name=f"{pfx}o{th}")
                    for hh in range(2):
                        nc.vector.tensor_copy(out=o_sb[:, hh * 512 : (hh + 1) * 512], in_=accs[th][hh])
                    nc.sync.dma_start(out=out_d[th * 128 : (th + 1) * 128, :], in_=o_sb)
                s_sb = opool.tile([1, T], F32, tag="s", name=f"{pfx}s")
                nc.vector.tensor_copy(out=s_sb, in_=s_ps)
                nc.sync.dma_start(out=s_out_d[:, :], in_=s_sb)

            phase(EP_SH, weT_d, epm_d, be_sb, epo_d, eps_d, "e")
            phase(SEM_SH, wsT_d, smm_d, bs_sb, smo_d, sms_d, "s")

    nc.compile()
    return nc


_NC_CACHE = {}


def _get_nc():
    if STREAM_DT not in _NC_CACHE:
        _NC_CACHE[STREAM_DT] = _build_bass()
    return _NC_CACHE[STREAM_DT]


def kernel(x, We, be, ep_mem, Ws, bs, sem_mem, Wc, bc, trace=False):
    x = np.asarray(x, np.float32)
    We = np.asarray(We, np.float32)
    be = np.asarray(be, np.float32)
    ep_mem = np.asarray(ep_mem, np.float32)
    Ws = np.asarray(Ws, np.float32)
    bs = np.asarray(bs, np.float32)
    sem_mem = np.asarray(sem_mem, np.float32)
    Wc = np.asarray(Wc, np.float32)
    bc = np.asarray(bc, np.float32)

    _, npdt, _ = _CFG[STREAM_DT]
    xf = x.reshape(T, H)
    xT = np.ascontiguousarray(xf.T).astype(npdt)
    WeT = np.ascontiguousarray(We.T)  # [H, EP]
    WsT = np.ascontiguousarray(Ws.T)  # [H, SEM]

    in_maps = []
    for i in range(NCORES):
        esl = slice(i * EP_SH, (i + 1) * EP_SH)
        ssl = slice(i * SEM_SH, (i + 1) * SEM_SH)
        in_maps.append(
            {
                "xT": xT,
                "weT": np.ascontiguousarray(WeT[:, esl]).astype(npdt),
                "be": np.ascontiguousarray(be[esl]),
                "epm": np.ascontiguousarray(ep_mem[esl]).astype(npdt),
                "wsT": np.ascontiguousarray(WsT[:, ssl]).astype(npdt),
                "bs": np.ascontiguousarray(bs[ssl]),
                "smm": np.ascontiguousarray(sem_mem[ssl]).astype(npdt),
            }
        )

    nc = _get_nc()
    res = run_bass_kernel_spmd(nc, in_maps, core_ids=list(range(NCORES)), trace=trace)

    ep_num = np.zeros((T, H), np.float64)
    ep_den = np.zeros((T,), np.float64)
    sm_num = np.zeros((T, H), np.float64)
    sm_den = np.zeros((T,), np.float64)
    for r in res.results:
        ep_num += r["ep_part"]
        ep_den += r["ep_s"].reshape(T)
        sm_num += r["sem_part"]
        sm_den += r["sem_s"].reshape(T)
    episodic = (ep_num / ep_den[:, None]).astype(np.float32)
    semantic = (sm_num / sm_den[:, None]).astype(np.float32)

    consolidated = np.concatenate([episodic, xf], axis=1)  # [T, 2H]
    out = consolidated @ Wc.T + bc

    out = out.reshape(B, S, H).astype(np.float32)
    semantic = semantic.reshape(B, S, H)
    if trace:
        return (out, semantic), res
    return out, semantic


# revision 10
# speedup vs baseline: 1.0599x; 1.0599x over previous
"""Trainium2 Bass kernel for the scatter-memory transformer block.

Computation (see harness reference):
    ep_w  = softmax(x @ We.T + be)   over 65536 slots
    episodic = ep_w @ ep_mem
    sem_w = softmax(x @ Ws.T + bs)   over 131072 slots
    semantic = sem_w @ sem_mem
    out = concat([episodic, x]) @ Wc.T + bc
    return (out, semantic)

Strategy: shard the slot axis across 8 NeuronCores (sequence-parallel flash
cross-attention over a fixed KV set).  Each core streams its slot shard once
from HBM (memory-bound: 192 MB/core), computing
    p[e, t]   = exp(W[e] . x[t] + b[e])          (no max subtraction -- logits
                                                  are O(0.2) for this model)
    part[t,h] = sum_e p[e, t] * mem[e, h]        (PSUM accumulation)
    s[t]      = sum_e p[e, t]
The host sums the 8 unnormalized partials, normalizes, and applies the tiny
consolidation linear.
"""

import numpy as np

import concourse.bass as bass
import concourse.mybir as mybir
import concourse.tile as tile
from concourse import bacc
from concourse.bass_utils import run_bass_kernel_spmd

# Problem dims (hardcoded per harness contract).
B, S, H = 2, 128, 1024
T = B * S  # 256 query tokens
EP, SEM = 65536, 131072
NCORES = 8
EP_SH = EP // NCORES  # 8192 episodic slots per core
SEM_SH = SEM // NCORES  # 16384 semantic slots per core
KH = H // 128  # 8 contraction chunks of 128

F32 = mybir.dt.float32

# Precision of the streamed operands (projections, memory banks, x, p):
# "fp32" is exact-ish (~1e-6 rel err); "fp16" halves HBM traffic
# (~2e-4 rel err from input quantization; accumulation stays fp32 in PSUM).
STREAM_DT = "fp32"
_CFG = {
    "fp32": (mybir.dt.float32, np.float32, 512),
    "fp16": (mybir.dt.float16, np.float16, 1024),
}


def _build_bass():
    SDT, _, CHUNK = _CFG[STREAM_DT]
    nc = bacc.Bacc(
        "TRN2",
        target_bir_lowering=False,
        debug=False,
        num_devices=NCORES,
    )

    xT_d = nc.dram_tensor("xT", [H, T], SDT, kind="ExternalInput")
    weT_d = nc.dram_tensor("weT", [H, EP_SH], SDT, kind="ExternalInput")
    be_d = nc.dram_tensor("be", [EP_SH], F32, kind="ExternalInput")
    epm_d = nc.dram_tensor("epm", [EP_SH, H], SDT, kind="ExternalInput")
    wsT_d = nc.dram_tensor("wsT", [H, SEM_SH], SDT, kind="ExternalInput")
    bs_d = nc.dram_tensor("bs", [SEM_SH], F32, kind="ExternalInput")
    smm_d = nc.dram_tensor("smm", [SEM_SH, H], SDT, kind="ExternalInput")

    epo_d = nc.dram_tensor("ep_part", [T, H], F32, kind="ExternalOutput")
    eps_d = nc.dram_tensor("ep_s", [1, T], F32, kind="ExternalOutput")
    smo_d = nc.dram_tensor("sem_part", [T, H], F32, kind="ExternalOutput")
    sms_d = nc.dram_tensor("sem_s", [1, T], F32, kind="ExternalOutput")

    with tile.TileContext(nc) as tc:
        with (
            tc.tile_pool(name="const", bufs=1) as cpool,
            tc.tile_pool(name="wstream", bufs=2) as wpool,
            tc.tile_pool(name="mstream", bufs=2) as mpool,
            tc.tile_pool(name="ptile", bufs=3) as ppool,
            tc.tile_pool(name="outp", bufs=2) as opool,
            tc.tile_pool(name="acc", bufs=1, space="PSUM") as acc_pool,
            tc.tile_pool(name="lg", bufs=2, space="PSUM") as lg_pool,
            tc.tile_pool(name="sm", bufs=1, space="PSUM") as sm_pool,
        ):
            # x.T resident in SBUF, laid out [p, k, t] with h = k*128 + p.
            xT_sb = cpool.tile([128, KH, T], SDT)
            nc.sync.dma_start(out=xT_sb, in_=xT_d[:, :].rearrange("(k p) t -> p k t", p=128))
            ones_sb = cpool.tile([128, 1], SDT)
            nc.vector.memset(ones_sb, 1.0)
            # Per-slot biases, [p, j] with slot = j*128 + p.
            be_sb = cpool.tile([128, EP_SH // 128], F32)
            nc.sync.dma_start(out=be_sb, in_=be_d[:].rearrange("(j p) -> p j", p=128))
            bs_sb = cpool.tile([128, SEM_SH // 128], F32)
            nc.sync.dma_start(out=bs_sb, in_=bs_d[:].rearrange("(j p) -> p j", p=128))

            def phase(n_sh, wT_d, mem_d, b_sb, out_d, s_out_d, pfx):
                n_chunks = n_sh // CHUNK
                jc = CHUNK // 128  # 128-slot subtiles per chunk
                accs = [
                    [
                        acc_pool.tile([128, 512], F32, tag=f"acc{th}{hh}", name=f"{pfx}acc{th}{hh}")
                        for hh in range(2)
                    ]
                    for th in range(2)
                ]
                s_ps = sm_pool.tile([1, T], F32, tag="sums", name=f"{pfx}sums")

                for c in range(n_chunks):
                    e0 = c * CHUNK
                    wt = wpool.tile([128, KH, CHUNK], SDT, tag="wt", name=f"{pfx}wt{c}")
                    nc.sync.dma_start(
                        out=wt, in_=wT_d[:, e0 : e0 + CHUNK].rearrange("(k p) e -> p k e", p=128)
                    )
                    mm = mpool.tile([128, jc, H], SDT, tag="mm", name=f"{pfx}mm{c}")
                    nc.sync.dma_start(
                        out=mm, in_=mem_d[e0 : e0 + CHUNK, :].rearrange("(j p) h -> p j h", p=128)
                    )
                    for j in range(jc):
                        # logits tile [128 slots, 256 tokens]
                        lp = lg_pool.tile([128, T], F32, tag="lg", name=f"{pfx}lg{c}_{j}")
                        for k in range(KH):
                            nc.tensor.matmul(
                                lp,
                                wt[:, k, j * 128 : (j + 1) * 128],
                                xT_sb[:, k, :],
                                start=(k == 0),
                                stop=(k == KH - 1),
                            )
                        # p = exp(l + b); stream q = p - 1 at SDT so the fp16
                        # quantization rides on the 0.18-scale fluctuation, not
                        # the unit-scale softmax weight.  Host adds back the
                        # exact uniform component (column sums of mem, fp64).
                        p32_sb = ppool.tile([128, T], F32, tag="p32", name=f"{pfx}p32_{c}_{j}")
                        gj = e0 // 128 + j
                        nc.scalar.activation(
                            out=p32_sb,
                            in_=lp,
                            func=mybir.ActivationFunctionType.Exp,
                            bias=b_sb[:, gj : gj + 1],
                            scale=1.0,
                        )
                        p_sb = ppool.tile([128, T], SDT, tag="p", name=f"{pfx}p{c}_{j}")
                        nc.vector.tensor_scalar_add(p_sb, p32_sb, -1.0)
                        first = c == 0 and j == 0
                        last = c == n_chunks - 1 and j == jc - 1
                        for th in range(2):
                            for hh in range(2):
                                nc.tensor.matmul(
                                    accs[th][hh],
                                    p_sb[:, th * 128 : (th + 1) * 128],
                                    mm[:, j, hh * 512 : (hh + 1) * 512],
                                    start=first,
                                    stop=last,
                                )
                        nc.tensor.matmul(s_ps, ones_sb, p_sb, start=first, stop=last)

                for th in range(2):
                    o_sb = opool.tile([128, H], F32, tag=f"o{th}", name=f"{pfx}o{th}")
                    for hh in range(2):
                        nc.vector.tensor_copy(out=o_sb[:, hh * 512 : (hh + 1) * 512], in_=accs[th][hh])
                    nc.sync.dma_start(out=out_d[th * 128 : (th + 1) * 128, :], in_=o_sb)
                s_sb = opool.tile([1, T], F32, tag="s", name=f"{pfx}s")
                nc.vector.tensor_copy(out=s_sb, in_=s_ps)
                nc.sync.dma_start(out=s_out_d[:, :], in_=s_sb)

            phase(EP_SH, weT_d, epm_d, be_sb, epo_d, eps_d, "e")
            phase(SEM_SH, wsT_d, smm_d, bs_sb, smo_d, sms_d, "s")

    nc.compile()
    return nc


_NC_CACHE = {}


def _get_nc():
    if STREAM_DT not in _NC_CACHE:
        _NC_CACHE[STREAM_DT] = _build_bass()
    return _NC_CACHE[STREAM_DT]


def kernel(x, We, be, ep_mem, Ws, bs, sem_mem, Wc, bc, trace=False):
    x = np.asarray(x, np.float32)
    We = np.asarray(We, np.float32)
    be = np.asarray(be, np.float32)
    ep_mem = np.asarray(ep_mem, np.float32)
    Ws = np.asarray(Ws, np.float32)
    bs = np.asarray(bs, np.float32)
    sem_mem = np.asarray(sem_mem, np.float32)
    Wc = np.asarray(Wc, np.float32)
    bc = np.asarray(bc, np.float32)

    _, npdt, _ = _CFG[STREAM_DT]
    xf = x.reshape(T, H)
    xT = np.ascontiguousarray(xf.T).astype(npdt)
    WeT = np.ascontiguousarray(We.T)  # [H, EP]
    WsT = np.ascontiguousarray(Ws.T)  # [H, SEM]

    in_maps = []
    for i in range(NCORES):
        esl = slice(i * EP_SH, (i + 1) * EP_SH)
        ssl = slice(i * SEM_SH, (i + 1) * SEM_SH)
        in_maps.append(
            {
                "xT": xT,
                "weT": np.ascontiguousarray(WeT[:, esl]).astype(npdt),
                "be": np.ascontiguousarray(be[esl]),
                "epm": np.ascontiguousarray(ep_mem[esl]).astype(npdt),
                "wsT": np.ascontiguousarray(WsT[:, ssl]).astype(npdt),
                "bs": np.ascontiguousarray(bs[ssl]),
                "smm": np.ascontiguousarray(sem_mem[ssl]).astype(npdt),
            }
        )

    nc = _get_nc()
    res = run_bass_kernel_spmd(nc, in_maps, core_ids=list(range(NCORES)), trace=trace)

    # Device partials hold sum_e q_e*mem[e] and sum_e q_e with q = p - 1;
    # add back the exact uniform component: sum_e mem[e] and the slot count.
    ep_num = ep_mem.sum(axis=0, dtype=np.float64)[None, :].repeat(T, 0)
    ep_den = np.full((T,), float(EP), np.float64)
    sm_num = sem_mem.sum(axis=0, dtype=np.float64)[None, :].repeat(T, 0)
    sm_den = np.full((T,), float(SEM), np.float64)
    for r in res.results:
        ep_num += r["ep_part"]
        ep_den += r["ep_s"].reshape(T)
        sm_num += r["sem_part"]
        sm_den += r["sem_s"].reshape(T)
    episodic = (ep_num / ep_den[:, None]).astype(np.float32)
    semantic = (sm_num / sm_den[:, None]).astype(np.float32)

    consolidated = np.concatenate([episodic, xf], axis=1)  # [T, 2H]
    out = consolidated @ Wc.T + bc

    out = out.reshape(B, S, H).astype(np.float32)
    semantic = semantic.reshape(B, S, H)
    if trace:
        return (out, semantic), res
    return out, semantic


# revision 11
# speedup vs baseline: 1.1622x; 1.0965x over previous
"""Trainium2 Bass kernel for the scatter-memory transformer block.

Computation (see harness reference):
    ep_w  = softmax(x @ We.T + be)   over 65536 slots
    episodic = ep_w @ ep_mem
    sem_w = softmax(x @ Ws.T + bs)   over 131072 slots
    semantic = sem_w @ sem_mem
    out = concat([episodic, x]) @ Wc.T + bc
    return (out, semantic)

Strategy: shard the slot axis across 8 NeuronCores (sequence-parallel flash
cross-attention over a fixed KV set).  Each core streams its slot shard once
from HBM (memory-bound: 192 MB/core), computing
    p[e, t]   = exp(W[e] . x[t] + b[e])          (no max subtraction -- logits
                                                  are O(0.2) for this model)
    part[t,h] = sum_e p[e, t] * mem[e, h]        (PSUM accumulation)
    s[t]      = sum_e p[e, t]
The host sums the 8 unnormalized partials, normalizes, and applies the tiny
consolidation linear.
"""

import numpy as np

import concourse.bass as bass
import concourse.mybir as mybir
import concourse.tile as tile
from concourse import bacc
from concourse.bass_utils import run_bass_kernel_spmd

# Problem dims (hardcoded per harness contract).
B, S, H = 2, 128, 1024
T = B * S  # 256 query tokens
EP, SEM = 65536, 131072
NCORES = 8
EP_SH = EP // NCORES  # 8192 episodic slots per core
SEM_SH = SEM // NCORES  # 16384 semantic slots per core
KH = H // 128  # 8 contraction chunks of 128

F32 = mybir.dt.float32

# Precision of the streamed operands (projections, memory banks, x, p):
# "fp32" is exact-ish (~1e-6 rel err); "fp16" halves HBM traffic
# (~2e-4 rel err from input quantization; accumulation stays fp32 in PSUM).
STREAM_DT = "fp32"
_CFG = {
    "fp32": (mybir.dt.float32, np.float32, 512),
    "fp16": (mybir.dt.float16, np.float16, 1024),
}


def _build_bass():
    SDT, _, CHUNK = _CFG[STREAM_DT]
    nc = bacc.Bacc(
        "TRN2",
        target_bir_lowering=False,
        debug=False,
        num_devices=NCORES,
    )

    xT_d = nc.dram_tensor("xT", [H, T], SDT, kind="ExternalInput")
    weT_d = nc.dram_tensor("weT", [H, EP_SH], SDT, kind="ExternalInput")
    be_d = nc.dram_tensor("be", [EP_SH], F32, kind="ExternalInput")
    epm_d = nc.dram_tensor("epm", [EP_SH, H + 1], SDT, kind="ExternalInput")
    wsT_d = nc.dram_tensor("wsT", [H, SEM_SH], SDT, kind="ExternalInput")
    bs_d = nc.dram_tensor("bs", [SEM_SH], F32, kind="ExternalInput")
    smm_d = nc.dram_tensor("smm", [SEM_SH, H + 1], SDT, kind="ExternalInput")

    epo_d = nc.dram_tensor("ep_part", [T, H], F32, kind="ExternalOutput")
    eps_d = nc.dram_tensor("ep_s", [T, 1], F32, kind="ExternalOutput")
    smo_d = nc.dram_tensor("sem_part", [T, H], F32, kind="ExternalOutput")
    sms_d = nc.dram_tensor("sem_s", [T, 1], F32, kind="ExternalOutput")

    with tile.TileContext(nc) as tc:
        with (
            tc.tile_pool(name="const", bufs=1) as cpool,
            tc.tile_pool(name="wstream", bufs=3) as wpool,
            tc.tile_pool(name="mstream", bufs=3) as mpool,
            tc.tile_pool(name="ptile", bufs=3) as ppool,
            tc.tile_pool(name="outp", bufs=2) as opool,
            tc.tile_pool(name="acc", bufs=1, space="PSUM") as acc_pool,
            tc.tile_pool(name="lg", bufs=2, space="PSUM") as lg_pool,
        ):
            # x.T resident in SBUF, laid out [p, k, t] with h = k*128 + p.
            xT_sb = cpool.tile([128, KH, T], SDT)
            nc.sync.dma_start(out=xT_sb, in_=xT_d[:, :].rearrange("(k p) t -> p k t", p=128))
            # Per-slot biases, [p, j] with slot = j*128 + p.
            be_sb = cpool.tile([128, EP_SH // 128], F32)
            nc.sync.dma_start(out=be_sb, in_=be_d[:].rearrange("(j p) -> p j", p=128))
            bs_sb = cpool.tile([128, SEM_SH // 128], F32)
            nc.sync.dma_start(out=bs_sb, in_=bs_d[:].rearrange("(j p) -> p j", p=128))

            def phase(n_sh, wT_d, mem_d, b_sb, out_d, s_out_d, pfx):
                n_chunks = n_sh // CHUNK
                jc = CHUNK // 128  # 128-slot subtiles per chunk
                accs = [
                    [
                        acc_pool.tile([128, 512], F32, tag=f"acc{th}{hh}", name=f"{pfx}acc{th}{hh}")
                        for hh in range(2)
                    ]
                    for th in range(2)
                ]
                s_ps = [
                    acc_pool.tile([128, 1], F32, tag=f"qsum{th}", name=f"{pfx}qsum{th}")
                    for th in range(2)
                ]

                for c in range(n_chunks):
                    e0 = c * CHUNK
                    wt = wpool.tile([128, KH, CHUNK], SDT, tag="wt", name=f"{pfx}wt{c}")
                    nc.sync.dma_start(
                        out=wt, in_=wT_d[:, e0 : e0 + CHUNK].rearrange("(k p) e -> p k e", p=128)
                    )
                    mm = mpool.tile([128, jc, H + 1], SDT, tag="mm", name=f"{pfx}mm{c}")
                    nc.sync.dma_start(
                        out=mm, in_=mem_d[e0 : e0 + CHUNK, :].rearrange("(j p) h -> p j h", p=128)
                    )
                    for j in range(jc):
                        # logits tile [128 slots, 256 tokens]
                        lp = lg_pool.tile([128, T], F32, tag="lg", name=f"{pfx}lg{c}_{j}")
                        for k in range(KH):
                            nc.tensor.matmul(
                                lp,
                                wt[:, k, j * 128 : (j + 1) * 128],
                                xT_sb[:, k, :],
                                start=(k == 0),
                                stop=(k == KH - 1),
                            )
                        # p = exp(l + b); stream q = p - 1 at SDT so the fp16
                        # quantization rides on the 0.18-scale fluctuation, not
                        # the unit-scale softmax weight.  Host adds back the
                        # exact uniform component (column sums of mem, fp64).
                        p32_sb = ppool.tile([128, T], F32, tag="p32", name=f"{pfx}p32_{c}_{j}")
                        gj = e0 // 128 + j
                        nc.scalar.activation(
                            out=p32_sb,
                            in_=lp,
                            func=mybir.ActivationFunctionType.Exp,
                            bias=b_sb[:, gj : gj + 1],
                            scale=1.0,
                        )
                        p_sb = ppool.tile([128, T], SDT, tag="p", name=f"{pfx}p{c}_{j}")
                        nc.vector.tensor_scalar_add(p_sb, p32_sb, -1.0)
                        first = c == 0 and j == 0
                        last = c == n_chunks - 1 and j == jc - 1
                        for th in range(2):
                            for hh in range(2):
                                nc.tensor.matmul(
                                    accs[th][hh],
                                    p_sb[:, th * 128 : (th + 1) * 128],
                                    mm[:, j, hh * 512 : (hh + 1) * 512],
                                    start=first,
                                    stop=last,
                                )
                            nc.tensor.matmul(
                                s_ps[th],
                                p_sb[:, th * 128 : (th + 1) * 128],
                                mm[:, j, H : H + 1],
                                start=first,
                                stop=last,
                            )

                for th in range(2):
                    o_sb = opool.tile([128, H], F32, tag=f"o{th}", name=f"{pfx}o{th}")
                    for hh in range(2):
                        nc.vector.tensor_copy(out=o_sb[:, hh * 512 : (hh + 1) * 512], in_=accs[th][hh])
                    nc.sync.dma_start(out=out_d[th * 128 : (th + 1) * 128, :], in_=o_sb)
                for th in range(2):
                    s_sb = opool.tile([128, 1], F32, tag=f"s{th}", name=f"{pfx}s{th}")
                    nc.vector.tensor_copy(out=s_sb, in_=s_ps[th])
                    nc.sync.dma_start(out=s_out_d[th * 128 : (th + 1) * 128, :], in_=s_sb)

            phase(EP_SH, weT_d, epm_d, be_sb, epo_d, eps_d, "e")
            phase(SEM_SH, wsT_d, smm_d, bs_sb, smo_d, sms_d, "s")

    nc.compile()
    return nc


_NC_CACHE = {}


def _get_nc():
    if STREAM_DT not in _NC_CACHE:
        _NC_CACHE[STREAM_DT] = _build_bass()
    return _NC_CACHE[STREAM_DT]


def kernel(x, We, be, ep_mem, Ws, bs, sem_mem, Wc, bc, trace=False):
    x = np.asarray(x, np.float32)
    We = np.asarray(We, np.float32)
    be = np.asarray(be, np.float32)
    ep_mem = np.asarray(ep_mem, np.float32)
    Ws = np.asarray(Ws, np.float32)
    bs = np.asarray(bs, np.float32)
    sem_mem = np.asarray(sem_mem, np.float32)
    Wc = np.asarray(Wc, np.float32)
    bc = np.asarray(bc, np.float32)

    _, npdt, _ = _CFG[STREAM_DT]
    xf = x.reshape(T, H)
    xT = np.ascontiguousarray(xf.T).astype(npdt)
    WeT = np.ascontiguousarray(We.T)  # [H, EP]
    WsT = np.ascontiguousarray(Ws.T)  # [H, SEM]

    in_maps = []
    for i in range(NCORES):
        esl = slice(i * EP_SH, (i + 1) * EP_SH)
        ssl = slice(i * SEM_SH, (i + 1) * SEM_SH)
        in_maps.append(
            {
                "xT": xT,
                "weT": np.ascontiguousarray(WeT[:, esl]).astype(npdt),
                "be": np.ascontiguousarray(be[esl]),
                "epm": np.concatenate(
                    [ep_mem[esl], np.ones((EP_SH, 1), np.float32)], axis=1
                ).astype(npdt),
                "wsT": np.ascontiguousarray(WsT[:, ssl]).astype(npdt),
                "bs": np.ascontiguousarray(bs[ssl]),
                "smm": np.concatenate(
                    [sem_mem[ssl], np.ones((SEM_SH, 1), np.float32)], axis=1
                ).astype(npdt),
            }
        )

    nc = _get_nc()
    res = run_bass_kernel_spmd(nc, in_maps, core_ids=list(range(NCORES)), trace=trace)

    # Device partials hold sum_e q_e*mem[e] and sum_e q_e with q = p - 1;
    # add back the exact uniform component: sum_e mem[e] and the slot count.
    ep_num = ep_mem.sum(axis=0, dtype=np.float64)[None, :].repeat(T, 0)
    ep_den = np.full((T,), float(EP), np.float64)
    sm_num = sem_mem.sum(axis=0, dtype=np.float64)[None, :].repeat(T, 0)
    sm_den = np.full((T,), float(SEM), np.float64)
    for r in res.results:
        ep_num += r["ep_part"]
        ep_den += r["ep_s"].reshape(T)
        sm_num += r["sem_part"]
        sm_den += r["sem_s"].reshape(T)
    episodic = (ep_num / ep_den[:, None]).astype(np.float32)
    semantic = (sm_num / sm_den[:, None]).astype(np.float32)

    consolidated = np.concatenate([episodic, xf], axis=1)  # [T, 2H]
    out = consolidated @ Wc.T + bc

    out = out.reshape(B, S, H).astype(np.float32)
    semantic = semantic.reshape(B, S, H)
    if trace:
        return (out, semantic), res
    return out, semantic
